# revision 20
# baseline (speedup 1.0000x reference)
"""Trainium2 Bass kernel for CodebookConv1D (VQ-dequant + GPT2-Conv1D matmul).

Computation: W = codebook[indices].reshape(2048, 8192); out = x @ W + bias.
Sharding: tensor-parallel over out_features (8192 -> 8 cores x 1024 columns).
Each core:
  - gathers its (2048 x 1024) W shard from the f32 codebook in HBM via
    indirect (SWDGE) DMA, casts to bf16, keeps it resident in SBUF
  - streams x in 128-row m-tiles: straight DMA load -> DVE cast to bf16 ->
    fused xbar DMA transpose into (128k x 16kc x 128m) layout -> 32 matmuls
    (16 k-chunks x 2 PSUM halves of N=512) accumulated in PSUM -> DVE bias
    add -> DMA store.
"""

import sys

if "/opt/trn_rl_repo" not in sys.path:
    sys.path.insert(0, "/opt/trn_rl_repo")

import numpy as np

IN_F = 2048
OUT_F = 8192
K_CB = 4096
BLOCK = 8
N_CORES = 8
M_FULL = 8192          # 4*2048 tokens
N_PER = OUT_F // N_CORES          # 1024 out columns per core
NBLK_PER = N_PER // BLOCK         # 128 index blocks per row per core
KC = IN_F // 128                  # 16 k-chunks
CB_PAD = 64                       # padded codebook row: 64 f32 = 256B
NIDX_CHUNK = 128 * NBLK_PER       # gather indices per k-chunk

_CACHE = {}


def _emit_dma_gather(
    nc, mybir, out_ap, in_ap, idxs_ap, num_idxs, elem_size, elem_step
):
    """InstDMAGatherAnt with a sub-256B payload (allowed for non-transpose;
    bass.dma_gather's %256 assert only applies to transpose mode). The
    256B-granularity constraint is on the source row stride (elem_step)."""
    eng = nc.gpsimd
    _in_ap = eng.lower_ap_dma(in_ap, for_custom_bir_dma=True)
    _idxs_ap = eng.lower_ap(idxs_ap)
    _out_ap = eng.lower_ap(out_ap)
    stride_bytes = elem_step * mybir.dt.size(in_ap.dtype)
    assert stride_bytes % 256 == 0
    return eng.add_instruction(
        mybir.InstDMAGatherAnt(
            name=nc.get_next_instruction_name(),
            ins=[*_in_ap, _idxs_ap, eng.lower_val_access(eng.to_reg(num_idxs))],
            outs=[_out_ap],
            transpose=False,
            num_idxs=num_idxs,
            elem_size=elem_size,
            stride_bytes_256=stride_bytes // 256,
            gen_mode=0,
            single_packet=True,
            queue_num=0,
            sbuf_tokens_per_rank=0,
            sbuf_free_dim_per_rank=0,
            sbuf_free_dim_pad_per_rank=0,
            sbuf_byte_offset=0,
        )
    )


def _build(n_mtiles):
    import concourse.bass as bass
    import concourse.bacc as bacc
    import concourse.mybir as mybir
    import concourse.tile as tile

    f32 = mybir.dt.float32
    bf16 = mybir.dt.bfloat16
    i32 = mybir.dt.int32
    m_rows = n_mtiles * 128

    nc = bacc.Bacc("TRN2", target_bir_lowering=False, num_swdge_queues=4)
    # x is pre-transposed on the host: xt[k, m] = x[m, k]
    xt_d = nc.dram_tensor("xt", [IN_F, m_rows], f32, kind="ExternalInput")
    cb_d = nc.dram_tensor("cb", [K_CB, BLOCK], f32, kind="ExternalInput")
    idx_d = nc.dram_tensor("idx", [IN_F, NBLK_PER], i32, kind="ExternalInput")
    bias_d = nc.dram_tensor("bias", [1, N_PER], f32, kind="ExternalInput")
    out_d = nc.dram_tensor("out", [m_rows, N_PER], f32, kind="ExternalOutput")

    with tile.TileContext(nc) as tc:
        with (
            tc.tile_pool(name="const", bufs=1) as constp,
            tc.tile_pool(name="wpool", bufs=1) as wpool,
            tc.tile_pool(name="stage", bufs=2) as stagep,
            tc.tile_pool(name="idxp", bufs=3) as idxp,
            tc.tile_pool(name="xio", bufs=3) as xio,
            tc.tile_pool(name="xbp", bufs=3) as xbp,
            tc.tile_pool(name="outp", bufs=3) as outp,
            tc.tile_pool(name="psum", bufs=4, space="PSUM") as psump,
        ):
            # --- constants: indices, bias ---
            idx_t = constp.tile([128, KC, NBLK_PER], i32)
            nc.gpsimd.dma_start(
                out=idx_t[:],
                in_=idx_d.rearrange("(kc p) b -> p kc b", p=128),
            )
            bias_t = constp.tile([128, N_PER], f32)
            nc.sync.dma_start(
                out=bias_t[:], in_=bias_d[:, :].to_broadcast([128, N_PER])
            )

            # --- gather W shard from codebook, cast to bf16, keep resident ---
            # HW indirect DMA honors ONE offset per partition: each gather
            # fills one 8-wide block column across all 128 k-partitions.
            # Spread across 4 SWDGE queues to parallelize Q7 descriptor gen.
            w_all = wpool.tile([128, KC, N_PER], bf16)
            for kc in range(KC):
                stage = stagep.tile([128, NBLK_PER, BLOCK], f32, tag="stage")
                for b in range(NBLK_PER):
                    inst = nc.gpsimd.indirect_dma_start(
                        out=stage[:, b, :],
                        out_offset=None,
                        in_=cb_d[:, :],
                        in_offset=bass.IndirectOffsetOnAxis(
                            ap=idx_t[:, kc, b : b + 1], axis=0
                        ),
                    )
                    q = b % 4
                    if q:
                        inst.ins.queue = f"qPoolDynamic{q}"
                nc.vector.tensor_copy(
                    out=w_all[:, kc, :],
                    in_=stage[:].rearrange("p g b -> p (g b)"),
                )

            # --- stream xT m-tiles (k on partitions, host pre-transposed) ---
            xt_r = xt_d.rearrange("(kc p) m -> p kc m", p=128)
            for mt in range(n_mtiles):
                xin = xio.tile([128, KC, 128], f32, tag="xin")
                nc.scalar.dma_start(
                    out=xin[:], in_=xt_r[:, :, mt * 128 : (mt + 1) * 128]
                )
                xb = xbp.tile([128, KC, 128], bf16, tag="xb")
                nc.vector.tensor_copy(out=xb[:], in_=xin[:])

                ps0 = psump.tile([128, 512], mybir.dt.float32, tag="ps")
                ps1 = psump.tile([128, 512], mybir.dt.float32, tag="ps")
                for kc in range(KC):
                    nc.tensor.matmul(
                        out=ps0[:],
                        lhsT=xb[:, kc, :],
                        rhs=w_all[:, kc, 0:512],
                        start=(kc == 0),
                        stop=(kc == KC - 1),
                    )
                    nc.tensor.matmul(
                        out=ps1[:],
                        lhsT=xb[:, kc, :],
                        rhs=w_all[:, kc, 512:1024],
                        start=(kc == 0),
                        stop=(kc == KC - 1),
                    )

                ot = outp.tile([128, N_PER], f32, tag="ot")
                nc.vector.tensor_tensor(
                    out=ot[:, 0:512],
                    in0=ps0[:],
                    in1=bias_t[:, 0:512],
                    op=mybir.AluOpType.add,
                )
                nc.vector.tensor_tensor(
                    out=ot[:, 512:1024],
                    in0=ps1[:],
                    in1=bias_t[:, 512:1024],
                    op=mybir.AluOpType.add,
                )
                nc.sync.dma_start(
                    out=out_d[mt * 128 : (mt + 1) * 128, :], in_=ot[:]
                )
    nc.compile()
    return nc


def get_nc(n_mtiles=M_FULL // 128):
    key = ("nc", n_mtiles)
    if key not in _CACHE:
        _CACHE[key] = _build(n_mtiles)
    return _CACHE[key]


def make_in_maps(x, codebook, indices, bias):
    """Host-side sharding: full inputs -> per-core input dicts."""
    xf = np.ascontiguousarray(
        np.asarray(x, dtype=np.float32).reshape(M_FULL, IN_F).T
    )
    cb = np.ascontiguousarray(np.asarray(codebook, dtype=np.float32))
    idx = np.asarray(indices, dtype=np.int32).reshape(IN_F, OUT_F // BLOCK)
    bias = np.asarray(bias, dtype=np.float32)
    in_maps = []
    for c in range(N_CORES):
        in_maps.append(
            {
                "xt": xf,
                "cb": cb,
                "idx": np.ascontiguousarray(
                    idx[:, c * NBLK_PER : (c + 1) * NBLK_PER]
                ),
                "bias": np.ascontiguousarray(
                    bias[c * N_PER : (c + 1) * N_PER]
                ).reshape(1, N_PER),
            }
        )
    return in_maps


def kernel(x, codebook, indices, bias):
    from concourse.bass_utils import run_bass_kernel_spmd

    nc = get_nc()
    in_maps = make_in_maps(x, codebook, indices, bias)
    res = run_bass_kernel_spmd(nc, in_maps, core_ids=list(range(N_CORES)))
    out = np.concatenate(
        [res.results[c]["out"] for c in range(N_CORES)], axis=1
    )
    return np.ascontiguousarray(out.reshape(4, 2048, OUT_F)).astype(
        np.float32, copy=False
    )
